# revision 2
# baseline (speedup 1.0000x reference)
"""AdaLoRA linear layer on 8 TRN2 NeuronCores.

Computes y = x @ (W + s * (P*Lambda*mask) @ Q)^T for
x[8192,4096], W[4096,4096], P[4096,64], Q[64,4096], s=2.0.

Strategy: data-parallel over the 8192 token dim (1024 tokens/core).
Each core computes its y shard with a single fused pass:
  t   = x_s @ Q^T                  (rank-64, tiny)
  y   = t_scaled @ P^T + x_s @ W^T (one PSUM accumulation group per tile)
All matmuls run in bf16 (f32 accumulation in PSUM); inputs are cast
f32->bf16 during the DMA into SBUF (SWDGE cast-DMA).

Host side passes transposed layouts (x^T, W^T, Q^T, P^T) so every DMA is
a natural contiguous load with the contraction dim on SBUF partitions.
"""

import os
import sys
import types

for _p in ("/opt/trn_rl_repo", "/opt/pypackages"):
    if os.path.isdir(_p) and _p not in sys.path:
        sys.path.append(_p)

# antenv.axon_hooks is not shipped in this image, but bass_utils imports it
# when trace=True under axon. Register a shim backed by the real ctypes hook.
if "antenv.axon_hooks" not in sys.modules:
    _mod = types.ModuleType("antenv.axon_hooks")
    _mod._hook = None

    def _set_hook(h, _m=_mod):
        _m._hook = h

    def _get_hook(_m=_mod):
        return _m._hook

    _mod.set_axon_ntff_profile_hook = _set_hook
    _mod.get_axon_ntff_profile_hook = _get_hook
    try:
        from trn_agent_boot.trn_boot import _ntff_profile_via_ctypes

        _mod._hook = _ntff_profile_via_ctypes("/opt/axon/libaxon_pjrt.so")
    except Exception:
        pass
    sys.modules["antenv.axon_hooks"] = _mod

import numpy as np

import concourse.bass as bass
import concourse.mybir as mybir
import concourse.tile as tile
from concourse import bacc
from concourse.bass_utils import run_bass_kernel_spmd
from concourse.masks import make_identity

N_CORES = 8
IN_F = 4096
OUT_F = 4096
RANK = 64
BT = 8192
M_PER = BT // N_CORES  # 1024 tokens per core
SCALING = 2.0

P_DIM = 128
KB = IN_F // P_DIM  # 32 k-blocks
MS = M_PER // P_DIM  # 8 m-subtiles per core
N_STRIPE = 512
NS = OUT_F // N_STRIPE  # 8 n-stripes

_graph_cache = None


def _build_graph():
    f32 = mybir.dt.float32
    bf16 = mybir.dt.bfloat16
    u8 = mybir.dt.uint8

    nc = bacc.Bacc(None, target_bir_lowering=False, debug=False)

    xt = nc.declare_dram_parameter("xt", [IN_F, M_PER], f32, isOutput=False)
    wt = nc.declare_dram_parameter("wt", [IN_F, OUT_F], f32, isOutput=False)
    qt = nc.declare_dram_parameter("qt", [IN_F, RANK], f32, isOutput=False)
    pt = nc.declare_dram_parameter("pt", [RANK, OUT_F], f32, isOutput=False)
    lam = nc.declare_dram_parameter("lam", [RANK, 1], f32, isOutput=False)
    mask = nc.declare_dram_parameter("mask", [RANK, 1], u8, isOutput=False)
    out = nc.declare_dram_parameter("out", [M_PER, OUT_F], f32, isOutput=True)

    xt_r = xt[:].rearrange("(kb p) m -> p kb m", p=P_DIM)
    wt_r = wt[:].rearrange("(kb p) n -> p kb n", p=P_DIM)
    qt_r = qt[:].rearrange("(kb p) r -> p kb r", p=P_DIM)

    with tile.TileContext(nc) as tc:
        with (
            tc.tile_pool(name="const", bufs=1) as constp,
            tc.tile_pool(name="xpool", bufs=1) as xpool,
            tc.tile_pool(name="wpool", bufs=2) as wpool,
            tc.tile_pool(name="ypool", bufs=3) as ypool,
            tc.tile_pool(name="tpool", bufs=2) as tpool,
            tc.tile_pool(name="psum_y", bufs=4, space="PSUM") as psum_y_pool,
            tc.tile_pool(name="psum_t", bufs=2, space="PSUM") as psum_t_pool,
        ):
            # ---- constants ----
            identity = constp.tile([P_DIM, P_DIM], bf16)
            make_identity(nc, identity)

            qt_sb = constp.tile([P_DIM, KB, RANK], bf16)
            nc.gpsimd.dma_start(out=qt_sb[:], in_=qt_r)

            # P^T zero-padded to 128 partitions (rank rows 64..127 are zero)
            pt_sb = constp.tile([P_DIM, OUT_F], bf16)
            nc.vector.memset(pt_sb[:], 0.0)
            nc.gpsimd.dma_start(out=pt_sb[0:RANK, :], in_=pt[:])

            lam_sb = constp.tile([RANK, 1], f32)
            nc.gpsimd.dma_start(out=lam_sb[:], in_=lam[:])
            mask_u8 = constp.tile([RANK, 1], u8)
            nc.gpsimd.dma_start(out=mask_u8[:], in_=mask[:])
            mask_f = constp.tile([RANK, 1], f32)
            nc.vector.tensor_copy(out=mask_f[:], in_=mask_u8[:])
            sv = constp.tile([RANK, 1], f32)
            nc.vector.tensor_mul(out=sv[:], in0=lam_sb[:], in1=mask_f[:])
            nc.scalar.mul(sv[:], sv[:], float(SCALING))

            # t^T (scaled), zero-padded to 128 partitions
            tT_all = constp.tile([P_DIM, MS, P_DIM], bf16)
            nc.vector.memset(tT_all[:], 0.0)

            # ---- x^T resident in SBUF (bf16), one tile per m-subtile ----
            xts = []
            for ms in range(MS):
                xt_ms = xpool.tile(
                    [P_DIM, KB, P_DIM], bf16, name=f"xt_ms{ms}", tag=f"xt_ms{ms}"
                )
                nc.gpsimd.dma_start(
                    out=xt_ms[:], in_=xt_r[:, :, ms * P_DIM : (ms + 1) * P_DIM]
                )
                xts.append(xt_ms)

            # ---- t = x @ Q^T, then tT_all[:, ms, :] = sv * t^T ----
            for ms in range(MS):
                psum_t = psum_t_pool.tile([P_DIM, RANK], f32, tag="psum_t")
                for kb in range(KB):
                    nc.tensor.matmul(
                        psum_t[:],
                        lhsT=xts[ms][:, kb, :],
                        rhs=qt_sb[:, kb, :],
                        start=(kb == 0),
                        stop=(kb == KB - 1),
                    )
                t_sb = tpool.tile([P_DIM, RANK], bf16, tag="t_sb")
                nc.any.tensor_copy(out=t_sb[:], in_=psum_t[:])
                psum_tT = psum_t_pool.tile([RANK, P_DIM], bf16, tag="psum_tT")
                nc.tensor.transpose(psum_tT[:], t_sb[:], identity[:])
                nc.scalar.mul(tT_all[0:RANK, ms, :], psum_tT[:], sv[:])

            # ---- main: y = t_scaled @ P^T + x @ W^T, stripe by stripe ----
            for ns in range(NS):
                wt_sb = wpool.tile([P_DIM, KB, N_STRIPE], bf16, tag="wt_sb")
                nc.gpsimd.dma_start(
                    out=wt_sb[:],
                    in_=wt_r[:, :, ns * N_STRIPE : (ns + 1) * N_STRIPE],
                )
                for ms in range(MS):
                    ypsum = psum_y_pool.tile([P_DIM, N_STRIPE], f32, tag="ypsum")
                    nc.tensor.matmul(
                        ypsum[:],
                        lhsT=tT_all[:, ms, :],
                        rhs=pt_sb[:, ns * N_STRIPE : (ns + 1) * N_STRIPE],
                        start=True,
                        stop=False,
                    )
                    for kb in range(KB):
                        nc.tensor.matmul(
                            ypsum[:],
                            lhsT=xts[ms][:, kb, :],
                            rhs=wt_sb[:, kb, :],
                            start=False,
                            stop=(kb == KB - 1),
                        )
                    y_sb = ypool.tile([P_DIM, N_STRIPE], f32, tag="y_sb")
                    nc.any.tensor_copy(out=y_sb[:], in_=ypsum[:])
                    nc.sync.dma_start(
                        out=out[
                            ms * P_DIM : (ms + 1) * P_DIM,
                            ns * N_STRIPE : (ns + 1) * N_STRIPE,
                        ],
                        in_=y_sb[:],
                    )

    nc.compile()
    return nc


def _get_graph():
    global _graph_cache
    if _graph_cache is None:
        _graph_cache = _build_graph()
    return _graph_cache


def run_full(inputs, trace=False, trace_kwargs=None):
    """Run the SPMD kernel on 8 cores. Returns (y_full, BassKernelResults)."""
    x = np.asarray(inputs["x"], dtype=np.float32)
    weight = np.asarray(inputs["weight"], dtype=np.float32)
    P = np.asarray(inputs["P"], dtype=np.float32)
    Lambda = np.asarray(inputs["Lambda"], dtype=np.float32)
    Q = np.asarray(inputs["Q"], dtype=np.float32)
    rank_mask = np.asarray(inputs["rank_mask"])

    xt = np.ascontiguousarray(x.T)  # [IN_F, BT]
    wt = np.ascontiguousarray(weight.T)  # [IN_F, OUT_F]
    qt = np.ascontiguousarray(Q.T)  # [IN_F, RANK]
    pt = np.ascontiguousarray(P.T)  # [RANK, OUT_F]
    lam = np.ascontiguousarray(Lambda.reshape(RANK, 1))
    mask_u8 = np.ascontiguousarray(rank_mask.reshape(RANK, 1).astype(np.uint8))

    in_maps = []
    for c in range(N_CORES):
        in_maps.append(
            {
                "xt": np.ascontiguousarray(xt[:, c * M_PER : (c + 1) * M_PER]),
                "wt": wt,
                "qt": qt,
                "pt": pt,
                "lam": lam,
                "mask": mask_u8,
            }
        )

    nc = _get_graph()
    res = run_bass_kernel_spmd(
        nc,
        in_maps,
        core_ids=list(range(N_CORES)),
        trace=trace,
        **(trace_kwargs or {}),
    )
    y = np.concatenate([res.results[c]["out"] for c in range(N_CORES)], axis=0)
    return y.astype(np.float32, copy=False), res


def kernel(**inputs) -> np.ndarray:
    y, _ = run_full(inputs, trace=False)
    return y


# revision 4
# speedup vs baseline: 1.0107x; 1.0107x over previous
"""AdaLoRA linear layer on 8 TRN2 NeuronCores.

Computes y = x @ (W + s * (P*Lambda*mask) @ Q)^T for
x[8192,4096], W[4096,4096], P[4096,64], Q[64,4096], s=2.0.

Strategy: data-parallel over the 8192 token dim (1024 tokens/core).
Each core computes its y shard with a single fused pass:
  t   = x_s @ Q^T                  (rank-64, tiny)
  y   = t_scaled @ P^T + x_s @ W^T (one PSUM accumulation group per tile)
All matmuls run in bf16 (f32 accumulation in PSUM); inputs are cast
f32->bf16 during the DMA into SBUF (SWDGE cast-DMA).

Host side passes transposed layouts (x^T, W^T, Q^T, P^T) so every DMA is
a natural contiguous load with the contraction dim on SBUF partitions.
"""

import os
import sys
import types

for _p in ("/opt/trn_rl_repo", "/opt/pypackages"):
    if os.path.isdir(_p) and _p not in sys.path:
        sys.path.append(_p)

# antenv.axon_hooks is not shipped in this image, but bass_utils imports it
# when trace=True under axon. Register a shim backed by the real ctypes hook.
if "antenv.axon_hooks" not in sys.modules:
    _mod = types.ModuleType("antenv.axon_hooks")
    _mod._hook = None

    def _set_hook(h, _m=_mod):
        _m._hook = h

    def _get_hook(_m=_mod):
        return _m._hook

    _mod.set_axon_ntff_profile_hook = _set_hook
    _mod.get_axon_ntff_profile_hook = _get_hook
    try:
        from trn_agent_boot.trn_boot import _ntff_profile_via_ctypes

        _mod._hook = _ntff_profile_via_ctypes("/opt/axon/libaxon_pjrt.so")
    except Exception:
        pass
    sys.modules["antenv.axon_hooks"] = _mod

import numpy as np

import concourse.bass as bass
import concourse.mybir as mybir
import concourse.tile as tile
from concourse import bacc
from concourse.bass_utils import run_bass_kernel_spmd
from concourse.tile_rust import add_dep_helper

N_CORES = 8
IN_F = 4096
OUT_F = 4096
RANK = 64
BT = 8192
M_PER = BT // N_CORES  # 1024 tokens per core
SCALING = 2.0

P_DIM = 128
KB = IN_F // P_DIM  # 32 k-blocks
MS = M_PER // P_DIM  # 8 m-subtiles per core
N_STRIPE = 512
NS = OUT_F // N_STRIPE  # 8 n-stripes

_graph_cache = None


def _build_graph():
    f32 = mybir.dt.float32
    bf16 = mybir.dt.bfloat16
    u8 = mybir.dt.uint8

    nc = bacc.Bacc(None, target_bir_lowering=False, debug=False)

    xt = nc.declare_dram_parameter("xt", [IN_F, M_PER], f32, isOutput=False)
    wt = nc.declare_dram_parameter("wt", [IN_F, OUT_F], f32, isOutput=False)
    qt = nc.declare_dram_parameter("qt", [IN_F, RANK], f32, isOutput=False)
    pt = nc.declare_dram_parameter("pt", [RANK, OUT_F], f32, isOutput=False)
    lam = nc.declare_dram_parameter("lam", [RANK, 1], f32, isOutput=False)
    mask = nc.declare_dram_parameter("mask", [RANK, 1], u8, isOutput=False)
    out = nc.declare_dram_parameter("out", [M_PER, OUT_F], f32, isOutput=True)

    xt_r = xt[:].rearrange("(kb p) m -> p kb m", p=P_DIM)
    wt_r = wt[:].rearrange("(kb p) n -> p kb n", p=P_DIM)
    qt_r = qt[:].rearrange("(kb p) r -> p kb r", p=P_DIM)

    XH = 512  # x chunk width (m); 2 chunks cover M_PER
    NXC = M_PER // XH

    with tile.TileContext(nc) as tc:
        with (
            tc.tile_pool(name="const", bufs=1) as constp,
            tc.tile_pool(name="xpool", bufs=1) as xpool,
            tc.tile_pool(name="wpool", bufs=2) as wpool,
            tc.tile_pool(name="ypool", bufs=3) as ypool,
            tc.tile_pool(name="psum_y", bufs=4, space="PSUM") as psum_y_pool,
            tc.tile_pool(name="psum_t", bufs=2, space="PSUM") as psum_t_pool,
        ):
            # ---- constants ----
            qt_sb = constp.tile([P_DIM, KB, RANK], bf16)
            dma_qt = nc.gpsimd.dma_start(out=qt_sb[:], in_=qt_r)

            # P^T zero-padded to 128 partitions (rank rows 64..127 are zero)
            pt_sb = constp.tile([P_DIM, OUT_F], bf16)
            nc.vector.memset(pt_sb[:], 0.0)
            dma_pt = nc.gpsimd.dma_start(out=pt_sb[0:RANK, :], in_=pt[:])

            lam_sb = constp.tile([RANK, 1], f32)
            nc.gpsimd.dma_start(out=lam_sb[:], in_=lam[:])
            mask_u8 = constp.tile([RANK, 1], u8)
            nc.gpsimd.dma_start(out=mask_u8[:], in_=mask[:])
            mask_f = constp.tile([RANK, 1], f32)
            nc.vector.tensor_copy(out=mask_f[:], in_=mask_u8[:])
            sv = constp.tile([RANK, 1], f32)
            nc.vector.tensor_mul(out=sv[:], in0=lam_sb[:], in1=mask_f[:])
            nc.scalar.mul(sv[:], sv[:], float(SCALING))

            # t^T (scaled) = sv * (x @ Q^T)^T, zero-padded to 128 partitions
            tT_all = constp.tile([P_DIM, NXC, XH], bf16)
            nc.vector.memset(tT_all[:], 0.0)

            # ---- x^T resident in SBUF (bf16), two m-chunks ----
            xts = []
            xdmas = []
            for h in range(NXC):
                xt_h = xpool.tile(
                    [P_DIM, KB, XH], bf16, name=f"xt_h{h}", tag=f"xt_h{h}"
                )
                d = nc.gpsimd.dma_start(
                    out=xt_h[:], in_=xt_r[:, :, h * XH : (h + 1) * XH]
                )
                xts.append(xt_h)
                xdmas.append(d)

            # ---- weight stripe DMAs (issued up front; ordered below) ----
            wts = []
            wdmas = []
            for ns in range(NS):
                wt_sb = wpool.tile([P_DIM, KB, N_STRIPE], bf16, tag="wt_sb")
                d = nc.gpsimd.dma_start(
                    out=wt_sb[:],
                    in_=wt_r[:, :, ns * N_STRIPE : (ns + 1) * N_STRIPE],
                )
                wts.append(wt_sb)
                wdmas.append(d)

            # DMA arrival order: (qt, x0) -> w0 -> (x1, pt) -> w1 -> w2 ... so
            # the PE's critical path (t(x0), then stripe-0 groups) unblocks
            # earliest instead of every transfer round-robining to the end.
            add_dep_helper(wdmas[0].ins, xdmas[0].ins, reason="w0 after x0")
            add_dep_helper(xdmas[1].ins, wdmas[0].ins, reason="x1 after w0")
            add_dep_helper(dma_pt.ins, wdmas[0].ins, reason="pt after w0")
            add_dep_helper(wdmas[1].ins, xdmas[1].ins, reason="w1 after x1")
            for ns in range(2, NS):
                add_dep_helper(
                    wdmas[ns].ins, wdmas[ns - 1].ins, reason=f"w{ns} chain"
                )

            def t_phase(h):
                # t^T[:, h*XH:(h+1)*XH] = sv * (x_h @ Q^T)^T  directly via
                # matmul with Q^T blocks stationary: out[r, m] in PSUM.
                psum_tT = psum_t_pool.tile([RANK, XH], f32, tag="psum_tT")
                for kb in range(KB):
                    nc.tensor.matmul(
                        psum_tT[:],
                        lhsT=qt_sb[:, kb, :],
                        rhs=xts[h][:, kb, :],
                        start=(kb == 0),
                        stop=(kb == KB - 1),
                    )
                nc.scalar.mul(tT_all[0:RANK, h, :], psum_tT[:], sv[:])

            def main_group(ns, ms):
                h, mo = divmod(ms, XH // P_DIM)
                msl = slice(mo * P_DIM, (mo + 1) * P_DIM)
                nsl = slice(ns * N_STRIPE, (ns + 1) * N_STRIPE)
                ypsum = psum_y_pool.tile([P_DIM, N_STRIPE], f32, tag="ypsum")
                for kb in range(KB):
                    nc.tensor.matmul(
                        ypsum[:],
                        lhsT=xts[h][:, kb, msl],
                        rhs=wts[ns][:, kb, :],
                        start=(kb == 0),
                        stop=False,
                    )
                nc.tensor.matmul(
                    ypsum[:],
                    lhsT=tT_all[:, h, msl],
                    rhs=pt_sb[:, nsl],
                    start=False,
                    stop=True,
                )
                y_sb = ypool.tile([P_DIM, N_STRIPE], f32, tag="y_sb")
                nc.any.tensor_copy(out=y_sb[:], in_=ypsum[:])
                nc.sync.dma_start(
                    out=out[ms * P_DIM : (ms + 1) * P_DIM, nsl], in_=y_sb[:]
                )

            # PE order: t(x0); stripe-0 groups for x0's tokens; t(x1);
            # stripe-0 groups for x1's tokens; then remaining stripes.
            t_phase(0)
            for ms in range(0, MS // 2):
                main_group(0, ms)
            t_phase(1)
            for ms in range(MS // 2, MS):
                main_group(0, ms)
            for ns in range(1, NS):
                for ms in range(MS):
                    main_group(ns, ms)

    nc.compile()
    return nc


def _get_graph():
    global _graph_cache
    if _graph_cache is None:
        _graph_cache = _build_graph()
    return _graph_cache


def run_full(inputs, trace=False, trace_kwargs=None):
    """Run the SPMD kernel on 8 cores. Returns (y_full, BassKernelResults)."""
    x = np.asarray(inputs["x"], dtype=np.float32)
    weight = np.asarray(inputs["weight"], dtype=np.float32)
    P = np.asarray(inputs["P"], dtype=np.float32)
    Lambda = np.asarray(inputs["Lambda"], dtype=np.float32)
    Q = np.asarray(inputs["Q"], dtype=np.float32)
    rank_mask = np.asarray(inputs["rank_mask"])

    xt = np.ascontiguousarray(x.T)  # [IN_F, BT]
    wt = np.ascontiguousarray(weight.T)  # [IN_F, OUT_F]
    qt = np.ascontiguousarray(Q.T)  # [IN_F, RANK]
    pt = np.ascontiguousarray(P.T)  # [RANK, OUT_F]
    lam = np.ascontiguousarray(Lambda.reshape(RANK, 1))
    mask_u8 = np.ascontiguousarray(rank_mask.reshape(RANK, 1).astype(np.uint8))

    in_maps = []
    for c in range(N_CORES):
        in_maps.append(
            {
                "xt": np.ascontiguousarray(xt[:, c * M_PER : (c + 1) * M_PER]),
                "wt": wt,
                "qt": qt,
                "pt": pt,
                "lam": lam,
                "mask": mask_u8,
            }
        )

    nc = _get_graph()
    res = run_bass_kernel_spmd(
        nc,
        in_maps,
        core_ids=list(range(N_CORES)),
        trace=trace,
        **(trace_kwargs or {}),
    )
    y = np.concatenate([res.results[c]["out"] for c in range(N_CORES)], axis=0)
    return y.astype(np.float32, copy=False), res


def kernel(**inputs) -> np.ndarray:
    y, _ = run_full(inputs, trace=False)
    return y
